# revision 1
# baseline (speedup 1.0000x reference)
"""Trainium2 Bass kernel for nn_DeformableAttention (B=4, C=384, H=W=56, NH=12, HC=32, STRIDE=2).

Self-contained: hardcodes shapes/sharding. Sharding: 8 cores = 4 batches x 2
pixel-row-halves. Each core computes the full value/key/offset branches for its
batch (duplicated across the pair) and the query branch + final GEMM for its
half of the 3136 output pixels.

Math note: the reference computes out = (scale * q^T k) v^T without softmax, so
attention is linear and reassociates:
    y[b] = (w_out @ blockdiag_h(scale * M[b,h])) @ Q[b],
    M[b,h] = V_s[b,h] K[b,h]^T  (32x32 per head)
which drops the 48x(3136x784x32) einsums to a few small GEMMs.
"""
import contextlib

import numpy as np

import concourse.bass as bass
import concourse.tile as tile
from concourse import bacc, mybir
from concourse.bass_utils import run_bass_kernel_spmd
from concourse.masks import make_identity

F32, F16, I32 = mybir.dt.float32, mybir.dt.float16, mybir.dt.int32
MULT, ADD, SUB = mybir.AluOpType.mult, mybir.AluOpType.add, mybir.AluOpType.subtract
AF = mybir.ActivationFunctionType

B, C, H, W = 4, 384, 56, 56
NH, HC = 12, 32
SCALE = HC ** -0.5
HP = H + 2                      # 58 padded
PIX = H * W                     # 3136
PIXPAD = 3200                   # padded to xbar 128-multiple
KH = KW = 28                    # stride-2 output
N = KH * KW                     # 784
NT = 112                        # point-tile size (7 tiles)
NTILES = N // NT
HALF_ROWS = H // 2              # 28
HALF_PIX = HALF_ROWS * W        # 1568
CT = C // 128                   # 3 channel tiles
EPS = 1e-5

_CACHE = {}


def _emit(nc, tc, ctx, io):
    pool = ctx.enter_context(tc.tile_pool(name="main", bufs=1))
    st32 = tc.tile_pool(name="stage32", bufs=1)
    st32p = st32.__enter__()
    dma = nc.sync

    # ---------------- loads ----------------
    xp32 = []
    for ct in range(CT):
        t = st32p.tile([128, HP * HP], F32, tag=f"xp32_{ct}")
        dma.dma_start(t[:], io["xp"][ct * 128:(ct + 1) * 128, :])
        xp32.append(t)
    xp16 = []
    for ct in range(CT):
        t = pool.tile([128, HP * HP], F16, tag=f"xp16_{ct}")
        nc.vector.tensor_copy(t[:], xp32[ct][:])
        xp16.append(t)
    xq16 = []
    for ct in range(CT):
        f = st32p.tile([128, 30 * HP], F32, tag=f"xq32_{ct}")
        dma.dma_start(f[:], io["xq"][ct * 128:(ct + 1) * 128, :])
        t = pool.tile([128, 30 * HP], F16, tag=f"xq16_{ct}")
        nc.vector.tensor_copy(t[:], f[:])
        xq16.append(t)

    def load_cols(name, width, dtype=F32):
        out = []
        for ct in range(CT):
            t = pool.tile([128, width], dtype, tag=f"{name}_{ct}")
            dma.dma_start(t[:], io[name][ct * 128:(ct + 1) * 128, :])
            out.append(t)
        return out

    wv = load_cols("wv", 9)
    wq = load_cols("wq", 9)
    wk = load_cols("wk", 9)
    wo = load_cols("wo", 9)
    bv = load_cols("bv", 1)
    bq = load_cols("bq", 1)
    bk = load_cols("bk", 1)
    bo = load_cols("bo", 1)
    lng = load_cols("lng", 1)
    lnb = load_cols("lnb", 1)
    w2t = load_cols("w2t", 2)
    wot32 = load_cols("wot", C)
    wot16 = []
    for ct in range(CT):
        t = pool.tile([128, C], F16, tag=f"wot16_{ct}")
        nc.vector.tensor_copy(t[:], wot32[ct][:])
        wot16.append(t)
    refyx = pool.tile([2, N], F32, tag="refyx")
    dma.dma_start(refyx[:], io["refyx"][:, :])
    ones = pool.tile([128, 1], F32, tag="ones")
    nc.vector.memset(ones[:], 1.0)
    ident = pool.tile([128, 128], F16, tag="ident")
    make_identity(nc, ident[:])

    # ---------------- conv helper ----------------
    def dwconv(eng, out2, xt, w, b, base_row, stride, rows, cols):
        # out2: [128, rows, cols] view; xt: [128, hp, 58] view (fp tile)
        for t in range(9):
            dy, dx = t // 3, t % 3
            r0 = base_row + dy
            src = xt[:, r0:r0 + (rows - 1) * stride + 1:stride,
                     dx:dx + (cols - 1) * stride + 1:stride]
            if t == 0:
                eng.tensor_scalar(out=out2, in0=src, scalar1=w[:, 0:1],
                                  scalar2=b[:, 0:1], op0=MULT, op1=ADD)
            else:
                eng.scalar_tensor_tensor(out=out2, in0=src, scalar=w[:, t:t + 1],
                                         in1=out2, op0=MULT, op1=ADD)

    # ---------------- off branch (fp32, critical path) ----------------
    off = []
    for ct in range(CT):
        t = pool.tile([128, N], F32, tag=f"off_{ct}")
        x3 = xp32[ct][:].rearrange("p (h w) -> p h w", h=HP)
        dwconv(nc.vector, t[:].rearrange("p (h w) -> p h w", h=KH),
               x3, wo[ct], bo[ct], 0, 2, KH, KW)
        off.append(t)
    st32.__exit__(None, None, None)

    with tc.tile_pool(name="ln_psum", bufs=1, space="PSUM") as lnp:
        mu_ps = lnp.tile([1, N], F32, tag="mu")
        ssq_ps = lnp.tile([1, N], F32, tag="ssq")
        sq = []
        for ct in range(CT):
            t = pool.tile([128, N], F32, tag=f"sq_{ct}")
            nc.scalar.activation(t[:], off[ct][:], AF.Square)
            sq.append(t)
        for sl in (slice(0, 512), slice(512, N)):
            for ct in range(CT):
                nc.tensor.matmul(mu_ps[:, sl], ones[:], off[ct][:, sl],
                                 start=(ct == 0), stop=(ct == CT - 1))
            for ct in range(CT):
                nc.tensor.matmul(ssq_ps[:, sl], ones[:], sq[ct][:, sl],
                                 start=(ct == 0), stop=(ct == CT - 1))
        # stats [1, N]
        mu = pool.tile([1, N], F32, tag="mu_sb")
        nc.scalar.activation(mu[:], mu_ps[:], AF.Copy, scale=1.0 / C)
        es = pool.tile([1, N], F32, tag="es_sb")
        nc.scalar.activation(es[:], ssq_ps[:], AF.Copy, scale=1.0 / C)
    musq = pool.tile([1, N], F32, tag="musq")
    nc.scalar.activation(musq[:], mu[:], AF.Square)
    var = pool.tile([1, N], F32, tag="var")
    nc.vector.tensor_tensor(out=var[:], in0=es[:], in1=musq[:], op=SUB)
    nc.vector.tensor_scalar_add(var[:], var[:], EPS)
    sd = pool.tile([1, N], F32, tag="sd")
    nc.scalar.activation(sd[:], var[:], AF.Sqrt)
    rstd = pool.tile([1, N], F32, tag="rstd")
    nc.vector.reciprocal(rstd[:], sd[:])
    # physically replicate mu/rstd across partitions via a K=1 PE matmul
    # (neither DVE nor DMA can broadcast-read a single partition)
    one_row = pool.tile([1, 128], F32, tag="one_row")
    nc.vector.memset(one_row[:], 1.0)
    mu_b = pool.tile([128, N], F32, tag="mu_b")
    rstd_b = pool.tile([128, N], F32, tag="rstd_b")
    with tc.tile_pool(name="bc_psum", bufs=1, space="PSUM") as bcp:
        bc_ps = bcp.tile([128, N], F32, tag="bc_ps")
        for sl in (slice(0, 512), slice(512, N)):
            nc.tensor.matmul(bc_ps[:, sl], one_row[:], mu[:, sl],
                             start=True, stop=True)
        nc.scalar.activation(mu_b[:], bc_ps[:], AF.Copy)
        for sl in (slice(0, 512), slice(512, N)):
            nc.tensor.matmul(bc_ps[:, sl], one_row[:], rstd[:, sl],
                             start=True, stop=True)
        nc.scalar.activation(rstd_b[:], bc_ps[:], AF.Copy)

    gel = []
    for ct in range(CT):
        t1 = sq[ct]  # reuse the square tile as scratch
        nc.vector.tensor_tensor(out=t1[:], in0=off[ct][:],
                                in1=mu_b[:], op=SUB)
        nc.vector.tensor_tensor(out=t1[:], in0=t1[:],
                                in1=rstd_b[:], op=MULT)
        nc.vector.tensor_scalar(out=t1[:], in0=t1[:], scalar1=lng[ct][:, 0:1],
                                scalar2=lnb[ct][:, 0:1], op0=MULT, op1=ADD)
        g = off[ct]  # reuse the off tile for the gelu output
        nc.scalar.activation(g[:], t1[:], AF.Gelu)
        gel.append(g)

    with tc.tile_pool(name="off_psum", bufs=1, space="PSUM") as offp:
        oyx_ps = offp.tile([2, N], F32, tag="oyx")
        for sl in (slice(0, 512), slice(512, N)):
            for ct in range(CT):
                nc.tensor.matmul(oyx_ps[:, sl], w2t[ct][:], gel[ct][:, sl],
                                 start=(ct == 0), stop=(ct == CT - 1))
        pos = pool.tile([2, N], F32, tag="pos")
        nc.vector.tensor_tensor(out=pos[:], in0=oyx_ps[:], in1=refyx[:], op=ADD)
    nc.scalar.activation(pos[:], pos[:], AF.Tanh)
    ixy = pool.tile([2, N], F32, tag="ixy")
    # iy/ix = (pos + 1) * (H-1)/2
    nc.vector.tensor_scalar(out=ixy[:], in0=pos[:], scalar1=(H - 1) / 2.0,
                            scalar2=(H - 1) / 2.0, op0=MULT, op1=ADD)
    ixy_write = dma.dma_start(io["ixy_dram"][:, :], ixy[:])

    # ---------------- value conv + pixel-major table ----------------
    val = []
    for ct in range(CT):
        t = pool.tile([128, PIXPAD], F16, tag=f"val_{ct}")
        nc.vector.memset(t[:, PIX:], 0.0)
        x3 = xp16[ct][:].rearrange("p (h w) -> p h w", h=HP)
        dwconv(nc.vector, t[:, :PIX].rearrange("p (h w) -> p h w", h=H),
               x3, wv[ct], bv[ct], 0, 1, H, W)
        val.append(t)
    vtab_writes = []
    with tc.tile_pool(name="vtp", bufs=3) as vtp:
        for chunk in range(PIXPAD // 128):
            wide = vtp.tile([128, C], F16, tag="vt_wide")
            for ct in range(CT):
                dma.dma_start_transpose(wide[:, ct * 128:(ct + 1) * 128],
                                        val[ct][:, chunk * 128:(chunk + 1) * 128])
            wi = dma.dma_start(io["vtab"][chunk * 128:(chunk + 1) * 128, :], wide[:])
            vtab_writes.append(wi)

    # ---------------- key conv + transpose ----------------
    key = []
    for ct in range(CT):
        t = pool.tile([128, N], F16, tag=f"key_{ct}")
        x3 = xp16[ct][:].rearrange("p (h w) -> p h w", h=HP)
        dwconv(nc.vector, t[:].rearrange("p (h w) -> p h w", h=KH),
               x3, wk[ct], bk[ct], 0, 2, KH, KW)
        key.append(t)
    kT = []
    with tc.tile_pool(name="ktp", bufs=2, space="PSUM") as ktp:
        for k in range(NTILES):
            t = pool.tile([NT, C], F16, tag=f"kT_{k}")
            for ct in range(CT):
                ps = ktp.tile([NT, 128], F16, tag="kt_ps", space="PSUM")
                nc.tensor.transpose(ps[:], key[ct][:, k * NT:(k + 1) * NT], ident[:])
                nc.scalar.activation(t[:, ct * 128:(ct + 1) * 128], ps[:], AF.Copy)
            kT.append(t)

    # ---------------- query conv ----------------
    q16 = []
    for ct in range(CT):
        t = pool.tile([128, HALF_PIX], F16, tag=f"q_{ct}")
        x3 = xq16[ct][:].rearrange("p (h w) -> p h w", h=30)
        dwconv(nc.vector, t[:].rearrange("p (h w) -> p h w", h=HALF_ROWS),
               x3, wq[ct], bq[ct], 0, 1, HALF_ROWS, W)
        q16.append(t)

    # ---------------- indices + gathers + bilinear ----------------
    vs = []
    with tc.tile_pool(name="gat", bufs=3) as gat:
        for k in range(NTILES):
            iy_x = gat.tile([NT, 2], F32, tag="iyx")
            # partition = point, free = (y,x)
            src = bass.AP(io["ixy_dram"].tensor, k * NT, [[1, NT], [N, 2]])
            rd = dma.dma_start(iy_x[:], src)
            tile.add_dep_helper(rd.ins, ixy_write.ins, reason="ixy dram RAW")
            xy0i = gat.tile([NT, 2], I32, tag="xy0i")
            nc.vector.tensor_copy(xy0i[:], iy_x[:])
            xy0f = gat.tile([NT, 2], F32, tag="xy0f")
            nc.vector.tensor_copy(xy0f[:], xy0i[:])
            # exact floor whether the int cast truncates (sim) or rounds (hw):
            # subtract 1 wherever cast result exceeds the input
            gtm = gat.tile([NT, 2], F32, tag="gtm")
            nc.vector.tensor_tensor(out=gtm[:], in0=xy0f[:], in1=iy_x[:],
                                    op=mybir.AluOpType.is_gt)
            nc.vector.tensor_tensor(out=xy0f[:], in0=xy0f[:], in1=gtm[:], op=SUB)
            nc.vector.tensor_scalar(out=xy0f[:], in0=xy0f[:], scalar1=float(H - 2),
                                    scalar2=None, op0=mybir.AluOpType.min)
            frac = gat.tile([NT, 2], F32, tag="frac")
            nc.vector.tensor_tensor(out=frac[:], in0=iy_x[:], in1=xy0f[:], op=SUB)
            omf = gat.tile([NT, 2], F32, tag="omf")
            nc.vector.tensor_scalar(out=omf[:], in0=frac[:], scalar1=-1.0,
                                    scalar2=1.0, op0=MULT, op1=ADD)
            # bilinear weights [NT,1] each: w00=(1-wy)(1-wx), w01=(1-wy)wx,
            # w10=wy(1-wx), w11=wy*wx   (col0=y, col1=x)
            wts = gat.tile([NT, 4], F32, tag="wts")
            nc.vector.tensor_tensor(out=wts[:, 0:1], in0=omf[:, 0:1], in1=omf[:, 1:2], op=MULT)
            nc.vector.tensor_tensor(out=wts[:, 1:2], in0=omf[:, 0:1], in1=frac[:, 1:2], op=MULT)
            nc.vector.tensor_tensor(out=wts[:, 2:3], in0=frac[:, 0:1], in1=omf[:, 1:2], op=MULT)
            nc.vector.tensor_tensor(out=wts[:, 3:4], in0=frac[:, 0:1], in1=frac[:, 1:2], op=MULT)
            idxf = gat.tile([NT, 1], F32, tag="idxf")
            nc.vector.scalar_tensor_tensor(out=idxf[:], in0=xy0f[:, 0:1], scalar=float(W),
                                           in1=xy0f[:, 1:2], op0=MULT, op1=ADD)
            idxi = gat.tile([NT, 1], I32, tag="idxi")
            nc.vector.tensor_copy(idxi[:], idxf[:])

            g = [gat.tile([NT, C], F16, tag=f"g{j}", name=f"g{j}_{k}") for j in range(4)]
            for j, delta in enumerate((0, 1, W, W + 1)):
                gi = nc.gpsimd.indirect_dma_start(
                    out=g[j][:], out_offset=None, in_=io["vtab"][:, :],
                    in_offset=bass.IndirectOffsetOnAxis(ap=idxi[:, :1], axis=0),
                    element_offset=delta * C,
                    bounds_check=PIX - 1, oob_is_err=False)
                for wi in vtab_writes:
                    tile.add_dep_helper(gi.ins, wi.ins, reason="vtab RAW")
            v = pool.tile([NT, C], F16, tag=f"vs_{k}")
            nc.vector.tensor_scalar(out=v[:], in0=g[0][:], scalar1=wts[:, 0:1],
                                    scalar2=None, op0=MULT)
            for j in range(1, 4):
                nc.vector.scalar_tensor_tensor(out=v[:], in0=g[j][:], scalar=wts[:, j:j + 1],
                                               in1=v[:], op0=MULT, op1=ADD)
            vs.append(v)

    # ---------------- M = V_s K^T per head (32x32), A^T, y ----------------
    # M via independent single-shot matmuls (the sim's psum zero-region
    # tracking is partition-blind, so multi-matmul accumulation groups from
    # different heads in one bank conflict); reduce the 7 k-slots on DVE.
    m16 = []
    with tc.tile_pool(name="mps", bufs=1, space="PSUM") as mps:
        m_ps = [mps.tile([128, HC * NTILES], F32, tag=f"m_ps{i}", name=f"m_ps{i}")
                for i in range(CT)]
        for h in range(NH):
            ct, j = h // 4, h % 4
            for k in range(NTILES):
                nc.tensor.matmul(m_ps[ct][j * 32:(j + 1) * 32, k * HC:(k + 1) * HC],
                                 vs[k][:, h * HC:(h + 1) * HC],
                                 kT[k][:, h * HC:(h + 1) * HC],
                                 start=True, stop=True,
                                 tile_position=(0, j * 32))
        for ct in range(CT):
            acc = pool.tile([128, HC], F32, tag=f"m32_{ct}")
            nc.scalar.activation(acc[:], m_ps[ct][:, 0:HC], AF.Copy)
            for k in range(1, NTILES):
                nc.vector.tensor_tensor(out=acc[:], in0=acc[:],
                                        in1=m_ps[ct][:, k * HC:(k + 1) * HC], op=ADD)
            t = pool.tile([128, HC], F16, tag=f"m16_{ct}")
            nc.scalar.activation(t[:], acc[:], AF.Copy, scale=SCALE)
            m16.append(t)

    at16 = []
    with tc.tile_pool(name="atps", bufs=1, space="PSUM") as atps:
        at_ps = [atps.tile([128, C], F32, tag=f"at_ps{i}", name=f"at_ps{i}") for i in range(CT)]
        for h in range(NH):
            ct, j = h // 4, h % 4
            nc.tensor.matmul(at_ps[ct][j * 32:(j + 1) * 32, :],
                             m16[ct][j * 32:(j + 1) * 32, :],
                             wot16[ct][j * 32:(j + 1) * 32, :],
                             start=True, stop=True,
                             tile_position=(j * 32, j * 32))
        for ct in range(CT):
            t = pool.tile([128, C], F16, tag=f"at16_{ct}")
            nc.scalar.activation(t[:], at_ps[ct][:], AF.Copy)
            at16.append(t)

    NCHUNK = 4
    CW = HALF_PIX // NCHUNK  # 392
    with tc.tile_pool(name="yps", bufs=2, space="PSUM") as yps, \
         tc.tile_pool(name="ysb", bufs=3) as ysb:
        for ot in range(CT):
            for ch in range(NCHUNK):
                y_ps = yps.tile([128, CW], F32, tag="y_ps", space="PSUM")
                for ct in range(CT):
                    nc.tensor.matmul(y_ps[:], at16[ct][:, ot * 128:(ot + 1) * 128],
                                     q16[ct][:, ch * CW:(ch + 1) * CW],
                                     start=(ct == 0), stop=(ct == CT - 1))
                y_sb = ysb.tile([128, CW], F32, tag="y_sb")
                nc.scalar.activation(y_sb[:], y_ps[:], AF.Copy)
                dma.dma_start(io["y"][ot * 128:(ot + 1) * 128, ch * CW:(ch + 1) * CW],
                              y_sb[:])


def build_program():
    if "nc" in _CACHE:
        return _CACHE["nc"]
    nc = bacc.Bacc("TRN2", target_bir_lowering=False, debug=False, num_devices=8)
    io = {}
    io["xp"] = nc.dram_tensor("xp", (C, HP * HP), F32, kind="ExternalInput").ap()
    io["xq"] = nc.dram_tensor("xq", (C, 30 * HP), F32, kind="ExternalInput").ap()
    for nm, shape in [("wv", (C, 9)), ("wq", (C, 9)), ("wk", (C, 9)), ("wo", (C, 9)),
                      ("bv", (C, 1)), ("bq", (C, 1)), ("bk", (C, 1)), ("bo", (C, 1)),
                      ("lng", (C, 1)), ("lnb", (C, 1)), ("w2t", (C, 2)),
                      ("wot", (C, C)), ("refyx", (2, N))]:
        io[nm] = nc.dram_tensor(nm, shape, F32, kind="ExternalInput").ap()
    io["vtab"] = nc.dram_tensor("vtab", (PIXPAD, C), F16).ap()
    io["ixy_dram"] = nc.dram_tensor("ixy_dram", (2, N), F32).ap()
    io["y"] = nc.dram_tensor("y", (C, HALF_PIX), F32, kind="ExternalOutput").ap()

    with tile.TileContext(nc) as tc:
        with contextlib.ExitStack() as ctx:
            _emit(nc, tc, ctx, io)
    nc.compile()
    _CACHE["nc"] = nc
    return nc


def host_prep(inputs):
    """Build the 8 per-core input maps from full inputs."""
    x = np.asarray(inputs["x"], np.float32)          # (B, C, H, W)
    xpad = np.pad(x, ((0, 0), (0, 0), (1, 1), (1, 1)))  # (B, C, 58, 58)
    shared = {}
    for nm, src in [("wv", "w_v"), ("wq", "w_q"), ("wk", "w_k"), ("wo", "w_off1")]:
        shared[nm] = np.asarray(inputs[src], np.float32).reshape(C, 9)
    for nm, src in [("bv", "b_v"), ("bq", "b_q"), ("bk", "b_k"), ("bo", "b_off1"),
                    ("lng", "ln_g"), ("lnb", "ln_b")]:
        shared[nm] = np.asarray(inputs[src], np.float32).reshape(C, 1)
    shared["w2t"] = np.ascontiguousarray(np.asarray(inputs["w_off2"], np.float32).T)  # (C,2)
    shared["wot"] = np.ascontiguousarray(np.asarray(inputs["w_out"], np.float32).T)   # (C,C) [c,o]
    ry = (np.arange(KH, dtype=np.float32) + 0.5) / KH * 2 - 1
    rx = (np.arange(KW, dtype=np.float32) + 0.5) / KW * 2 - 1
    refyx = np.stack([np.repeat(ry, KW), np.tile(rx, KH)])   # (2, 784), row0=y
    shared["refyx"] = np.ascontiguousarray(refyx, dtype=np.float32)

    in_maps = []
    for core in range(8):
        b, half = core // 2, core % 2
        m = dict(shared)
        m["xp"] = np.ascontiguousarray(xpad[b].reshape(C, HP * HP))
        r0 = half * HALF_ROWS
        m["xq"] = np.ascontiguousarray(xpad[b, :, r0:r0 + 30, :].reshape(C, 30 * HP))
        in_maps.append(m)
    return in_maps


def assemble(results):
    y = np.empty((B, C, H, W), np.float32)
    for core in range(8):
        b, half = core // 2, core % 2
        part = results[core]["y"].reshape(C, HALF_ROWS, W)
        y[b, :, half * HALF_ROWS:(half + 1) * HALF_ROWS, :] = part
    return y


def run(inputs, trace=False):
    nc = build_program()
    in_maps = host_prep(inputs)
    res = run_bass_kernel_spmd(nc, in_maps, core_ids=list(range(8)), trace=trace)
    return assemble(res.results), res


def kernel(**inputs):
    out, _ = run(inputs, trace=False)
    return out



# revision 17
# speedup vs baseline: 1.5855x; 1.5855x over previous
"""Trainium2 Bass kernel for nn_DeformableAttention (B=4, C=384, H=W=56, NH=12, HC=32, STRIDE=2).

Self-contained: hardcodes shapes/sharding. Sharding: 8 cores = 4 batches x 2
pixel-row-halves. Each core computes the full value/key/offset branches for its
batch (duplicated across the pair) and the query branch + final GEMM for its
half of the 3136 output pixels.

Math note: the reference computes out = (scale * q^T k) v^T without softmax, so
attention is linear and reassociates:
    y[b] = (w_out @ blockdiag_h(scale * M[b,h])) @ Q[b],
    M[b,h] = V_s[b,h] K[b,h]^T  (32x32 per head)
which drops the 48x(3136x784x32) einsums to a few small GEMMs.

v2 layout notes:
  - x is padded to 58x58 and convs run on the flat 58-wide raster, so every
    stride-1 tap is a contiguous 2D slice (output cols x=56,57 are garbage and
    never read; pixel index = y*58+x).
  - value conv runs on the PE as 9 diag-weight matmuls per 512-pixel PSUM
    chunk; chunks are PE-transposed into a pixel-major fp16 vtab in DRAM.
  - the 4 bilinear neighbors are fetched with 4 dma_gather ops (wrap-16 int16
    index layout, built via one-hot-column matmuls from the point-major index
    tile), giving point-major [128, 7, 384] tiles directly.
"""
import contextlib

import numpy as np

import concourse.bass as bass
import concourse.tile as tile
from concourse import bacc, mybir
from concourse.bass_utils import run_bass_kernel_spmd

F32, F16, I32, I16 = mybir.dt.float32, mybir.dt.float16, mybir.dt.int32, mybir.dt.int16
F32R = mybir.dt.float32r
MULT, ADD, SUB = mybir.AluOpType.mult, mybir.AluOpType.add, mybir.AluOpType.subtract
MIN = mybir.AluOpType.min
AF = mybir.ActivationFunctionType

B, C, H, W = 4, 384, 56, 56
NH, HC = 12, 32
SCALE = HC ** -0.5
HP = H + 2                      # 58: padded row width; flat pixel = y*58+x
XW = HP * HP + 2                # 3366: padded x cols (+2 so last tap stays in-bounds)
PIXF = H * HP                   # 3248 flat conv-output pixels (y in [0,56))
KH = KW = 28                    # stride-2 output
N = KH * KW                     # 784 offset points
NCH = 7                         # point chunks of 128 (last has 16 valid)
HALF_ROWS = H // 2              # 28
QF = HALF_ROWS * HP             # 1624 flat query cols per core
QXW = (HALF_ROWS + 2) * HP + 2  # 1742: per-core query input (30 rows + tap pad)
HALF_PIX = HALF_ROWS * W        # 1568 valid output pixels per core
CT = C // 128                   # 3 channel tiles
EPS = 1e-5
IMAX = float(H - 2)             # 54: floor clamp so idx+59 stays valid

_CACHE = {}


def _emit(nc, tc, ctx, io):
    pool = ctx.enter_context(tc.tile_pool(name="main", bufs=1))
    dma = nc.sync

    # ---------------- loads ----------------
    xh = []
    for ct in range(CT):
        t = pool.tile([128, XW], F16, tag=f"xh_{ct}")
        dma.dma_start(t[:], io["xh"][ct * 128:(ct + 1) * 128, :])
        xh.append(t)
    dv = pool.tile([128, 57 * 128], F16, tag="dv")
    dma.dma_start(dv[:], io["dv"][:, :])
    wp = []
    for ct in range(CT):
        t = pool.tile([128, 32], F32, tag=f"wp_{ct}")
        dma.dma_start(t[:], io["wpack"][ct * 128:(ct + 1) * 128, :])
        wp.append(t)
    # wpack cols: 0-8 wq, 9-17 wk, 18-26 wo, 27 bq, 28 bk, 29 bo, 30 lng, 31 lnb
    xq = []
    for ct in range(CT):
        t = pool.tile([128, QXW], F16, tag=f"xq_{ct}")
        dma.dma_start(t[:], io["xq"][ct * 128:(ct + 1) * 128, :])
        xq.append(t)
    w2 = []
    for ct in range(CT):
        t = pool.tile([128, 2], F32, tag=f"w2_{ct}")
        dma.dma_start(t[:], io["w2t"][ct * 128:(ct + 1) * 128, :])
        w2.append(t)
    wot16 = []
    for ct in range(CT):
        t = pool.tile([128, C], F16, tag=f"wot_{ct}")
        dma.dma_start(t[:], io["wot"][ct * 128:(ct + 1) * 128, :])
        wot16.append(t)
    bvT = pool.tile([1, C], F32, tag="bvT")
    dma.dma_start(bvT[:], io["bvT"][:, :])
    refyx = pool.tile([2, N], F32, tag="refyx")
    dma.dma_start(refyx[:], io["refyx"][:, :])
    identF = pool.tile([128, 128], F32, tag="identF")
    dma.dma_start(identF[:], io["identF"][:, :])
    one_row = pool.tile([1, 128], F32, tag="one_row")
    nc.vector.memset(one_row[:], 1.0)
    ones_col = pool.tile([128, 1], F32, tag="ones_col")
    nc.vector.memset(ones_col[:], 1.0)

    def conv_taps(eng, out2, src3, w, wcol, b, bcol, stride, rows, cols):
        # out2: [128, rows, cols] view; src3: [128, 58/60, 58] view
        for t in range(9):
            dy, dx = t // 3, t % 3
            src = src3[:, dy:dy + (rows - 1) * stride + 1:stride,
                       dx:dx + (cols - 1) * stride + 1:stride]
            if t == 0:
                eng.tensor_scalar(out=out2, in0=src, scalar1=w[:, wcol:wcol + 1],
                                  scalar2=b[:, bcol:bcol + 1], op0=MULT, op1=ADD)
            else:
                eng.scalar_tensor_tensor(out=out2, in0=src, scalar=w[:, wcol + t:wcol + t + 1],
                                         in1=out2, op0=MULT, op1=ADD)

    def flat_taps(eng, out_t, xt, w, wcol, b, bcol, r0, width):
        # stride-1 conv on the flat 58-raster: every tap is a contiguous slice
        for t in range(9):
            dy, dx = t // 3, t % 3
            off = (r0 + dy) * HP + dx
            src = xt[:, off:off + width]
            if t == 0:
                eng.tensor_scalar(out=out_t, in0=src, scalar1=w[:, wcol:wcol + 1],
                                  scalar2=b[:, bcol:bcol + 1], op0=MULT, op1=ADD)
            else:
                eng.scalar_tensor_tensor(out=out_t, in0=src, scalar=w[:, wcol + t:wcol + t + 1],
                                         in1=out_t, op0=MULT, op1=ADD)

    # ---------------- key conv (PE diag matmuls, stride-2 views) ----------------
    ones16 = pool.tile([128, 448], F16, tag="ones16")
    nc.vector.memset(ones16[:], 1.0)
    key = []
    KCH = ((0, 16), (16, 12))   # row-chunks of the 28x28 output
    with tc.tile_pool(name="kc_ps", bufs=2, space="PSUM") as kcp:
        for ct in range(CT):
            t = pool.tile([128, N], F16, tag=f"key_{ct}")
            x3 = xh[ct][:, :HP * HP].rearrange("p (h w) -> p h w", h=HP)
            for r0, rows in KCH:
                cw = rows * KW
                ps = kcp.tile([128, 448], F32, tag="kc_ps", space="PSUM")
                for tp in range(9):
                    dy, dx = tp // 3, tp % 3
                    src = x3[:, dy + 2 * r0:dy + 2 * r0 + 2 * rows - 1:2,
                             dx:dx + 2 * KW - 1:2]
                    nc.tensor.matmul(ps[:, :cw],
                                     dv[:, (27 + ct * 9 + tp) * 128:(28 + ct * 9 + tp) * 128],
                                     src, start=(tp == 0), stop=False)
                nc.tensor.matmul(ps[:, :cw], dv[:, (54 + ct) * 128:(55 + ct) * 128],
                                 ones16[:, :cw], start=False, stop=True)
                nc.scalar.activation(t[:, r0 * KW:r0 * KW + cw], ps[:, :cw], AF.Copy)
            key.append(t)

    # ---------------- off conv (DVE) + LayerNorm + GELU ----------------
    off = []
    for ct in range(CT):
        t = pool.tile([128, N], F32, tag=f"off_{ct}")
        x3 = xh[ct][:, :HP * HP].rearrange("p (h w) -> p h w", h=HP)
        conv_taps(nc.vector, t[:].rearrange("p (h w) -> p h w", h=KH),
                  x3, wp[ct], 18, wp[ct], 29, 2, KH, KW)
        off.append(t)

    sq = []
    for ct in range(CT):
        t = pool.tile([128, N], F32, tag=f"sq_{ct}")
        nc.scalar.activation(t[:], off[ct][:], AF.Square)
        sq.append(t)

    SLICES = (slice(0, 512), slice(512, N))
    with tc.tile_pool(name="ln_ps", bufs=1, space="PSUM") as lnp:
        mu_ps = lnp.tile([1, N], F32, tag="mu_ps")
        ssq_ps = lnp.tile([1, N], F32, tag="ssq_ps")
        for sl in SLICES:
            for ct in range(CT):
                nc.tensor.matmul(mu_ps[:, sl], ones_col[:, :],
                                 off[ct][:, sl],
                                 start=(ct == 0), stop=(ct == CT - 1))
            for ct in range(CT):
                nc.tensor.matmul(ssq_ps[:, sl], ones_col[:, :],
                                 sq[ct][:, sl],
                                 start=(ct == 0), stop=(ct == CT - 1))
        mu = pool.tile([1, N], F32, tag="mu_sb")
        nc.scalar.activation(mu[:], mu_ps[:], AF.Copy, scale=1.0 / C)
        es = pool.tile([1, N], F32, tag="es_sb")
        nc.scalar.activation(es[:], ssq_ps[:], AF.Copy, scale=1.0 / C)
    musq = pool.tile([1, N], F32, tag="musq")
    nc.scalar.activation(musq[:], mu[:], AF.Square)
    var = pool.tile([1, N], F32, tag="var")
    nc.vector.tensor_tensor(out=var[:], in0=es[:], in1=musq[:], op=SUB)
    nc.vector.tensor_scalar_add(var[:], var[:], EPS)
    sd = pool.tile([1, N], F32, tag="sd")
    nc.scalar.activation(sd[:], var[:], AF.Sqrt)
    rstd = pool.tile([1, N], F32, tag="rstd")
    nc.vector.reciprocal(rstd[:], sd[:])

    mu_b = pool.tile([128, N], F32, tag="mu_b")
    rstd_b = pool.tile([128, N], F32, tag="rstd_b")
    with tc.tile_pool(name="bc_ps", bufs=1, space="PSUM") as bcp:
        bc_ps = bcp.tile([128, 512], F32, tag="bc_ps")
        for src, dst in ((mu, mu_b), (rstd, rstd_b)):
            for sl in SLICES:
                w_ = sl.stop - sl.start
                nc.tensor.matmul(bc_ps[:, :w_], one_row[:, :],
                                 src[:, sl], start=True, stop=True)
                nc.scalar.activation(dst[:, sl], bc_ps[:, :w_], AF.Copy)

    gel = []
    for ct in range(CT):
        t1 = sq[ct]
        nc.vector.tensor_tensor(out=t1[:], in0=off[ct][:], in1=mu_b[:], op=SUB)
        nc.vector.tensor_tensor(out=t1[:], in0=t1[:], in1=rstd_b[:], op=MULT)
        nc.vector.tensor_scalar(out=t1[:], in0=t1[:], scalar1=wp[ct][:, 30:31],
                                scalar2=wp[ct][:, 31:32], op0=MULT, op1=ADD)
        g = off[ct]
        nc.scalar.activation(g[:], t1[:], AF.Gelu)
        gel.append(g)

    # ---------------- offsets -> point-major iy/ix [128, (7,2)] ----------------
    pos = pool.tile([2, N], F32, tag="pos")
    with tc.tile_pool(name="oyx_ps", bufs=1, space="PSUM") as oxp:
        o_ps = oxp.tile([2, N], F32, tag="o_ps")
        for sl in SLICES:
            for ct in range(CT):
                nc.tensor.matmul(o_ps[:, sl], w2[ct][:, :],
                                 gel[ct][:, sl],
                                 start=(ct == 0), stop=(ct == CT - 1))
        nc.vector.tensor_tensor(out=pos[:], in0=o_ps[:], in1=refyx[:], op=ADD)

    iyx = pool.tile([128, 14], F32, tag="iyx")
    with tc.tile_pool(name="iyx_ps", bufs=1, space="PSUM") as ixp:
        i_ps = ixp.tile([128, 14], F32, tag="i_ps")
        for k in range(NCH):
            kn = min(128, N - k * 128)
            nc.tensor.transpose(i_ps[:kn, 2 * k:2 * k + 2],
                                pos[:, k * 128:k * 128 + kn], identF[:2, :2])
        nc.scalar.activation(iyx[:], i_ps[:], AF.Tanh)
    # iy/ix = (tanh+1)*(H-1)/2
    nc.vector.tensor_scalar(out=iyx[:], in0=iyx[:], scalar1=(H - 1) / 2.0,
                            scalar2=(H - 1) / 2.0, op0=MULT, op1=ADD)

    # floor + clamp (exact floor whether the int cast truncates or rounds)
    xy0i = pool.tile([128, 14], I32, tag="xy0i")
    nc.vector.tensor_copy(xy0i[:], iyx[:])
    xy0f = pool.tile([128, 14], F32, tag="xy0f")
    nc.vector.tensor_copy(xy0f[:], xy0i[:])
    gtm = pool.tile([128, 14], F32, tag="gtm")
    nc.vector.tensor_tensor(out=gtm[:], in0=xy0f[:], in1=iyx[:], op=mybir.AluOpType.is_gt)
    nc.vector.tensor_tensor(out=xy0f[:], in0=xy0f[:], in1=gtm[:], op=SUB)
    nc.vector.tensor_scalar(out=xy0f[:], in0=xy0f[:], scalar1=IMAX, scalar2=None, op0=MIN)
    frac = pool.tile([128, 14], F32, tag="frac")
    nc.vector.tensor_tensor(out=frac[:], in0=iyx[:], in1=xy0f[:], op=SUB)
    omf = pool.tile([128, 14], F32, tag="omf")
    nc.vector.tensor_scalar(out=omf[:], in0=frac[:], scalar1=-1.0, scalar2=1.0,
                            op0=MULT, op1=ADD)

    # bilinear weights, point-major [128, (4j, 7k)]
    wts = pool.tile([128, 28], F32, tag="wts")
    f3 = frac[:].rearrange("p (k t) -> p k t", t=2)
    o3 = omf[:].rearrange("p (k t) -> p k t", t=2)
    nc.vector.tensor_tensor(out=wts[:, 0:7], in0=o3[:, :, 0], in1=o3[:, :, 1], op=MULT)
    nc.vector.tensor_tensor(out=wts[:, 7:14], in0=o3[:, :, 0], in1=f3[:, :, 1], op=MULT)
    nc.vector.tensor_tensor(out=wts[:, 14:21], in0=f3[:, :, 0], in1=o3[:, :, 1], op=MULT)
    nc.vector.tensor_tensor(out=wts[:, 21:28], in0=f3[:, :, 0], in1=f3[:, :, 1], op=MULT)
    # (pad points >= 784 only exist in chunk 6 partitions >= 16, which no
    # consumer reads: bilinear and M slice [:kn] there)

    # 4 gather index variants, point-major f32
    idx4 = pool.tile([128, 28], F32, tag="idx4")
    x3v = xy0f[:].rearrange("p (k t) -> p k t", t=2)
    nc.vector.scalar_tensor_tensor(out=idx4[:, 0:7], in0=x3v[:, :, 0], scalar=float(HP),
                                   in1=x3v[:, :, 1], op0=MULT, op1=ADD)
    for j, d in ((1, 1.0), (2, float(HP)), (3, float(HP + 1))):
        nc.vector.tensor_scalar(out=idx4[:, j * 7:j * 7 + 7], in0=idx4[:, 0:7],
                                scalar1=d, scalar2=None, op0=ADD)

    # wrap-16 int16 index layout for dma_gather: idxw[q, j, 8k+a] = idx_j of
    # point 128k+16a+q, built via one-hot-column matmuls + permuted-copy cast.
    idxw = pool.tile([128, 224], I16, tag="idxw")
    with tc.tile_pool(name="wr_ps", bufs=1, space="PSUM") as wrp:
        w_ps = wrp.tile([16, 224], F32, tag="w_ps")
        for a in range(8):
            nc.tensor.matmul(w_ps[:, a * 28:(a + 1) * 28],
                             identF[:, 16 * a:16 * (a + 1)], idx4[:, :],
                             start=True, stop=True)
        src = w_ps[:].rearrange("p (a v k) -> p v k a", a=8, v=4)
        dst = idxw[0:16, :].rearrange("p (v k a) -> p v k a", v=4, k=7)
        nc.vector.tensor_copy(dst, src)
    for lo, n_ in ((16, 16), (32, 32), (64, 64)):
        dma.dma_start(idxw[lo:lo + n_, :], idxw[0:n_, :])

    # ---------------- value conv (PE diag matmuls) + vtab ----------------
    val = []
    for ct in range(CT):
        t = pool.tile([128, PIXF], F16, tag=f"val_{ct}")
        val.append(t)
    CHUNKS = [(c * 512, min(512, PIXF - c * 512)) for c in range((PIXF + 511) // 512)]
    with tc.tile_pool(name="vc_ps", bufs=2, space="PSUM") as vcp:
        for ct in range(CT):
            for c0, cw in CHUNKS:
                ps = vcp.tile([128, 512], F32, tag="vc_ps", space="PSUM")
                for t in range(9):
                    dy, dx = t // 3, t % 3
                    off_t = dy * HP + dx
                    nc.tensor.matmul(ps[:, :cw], dv[:, (ct * 9 + t) * 128:(ct * 9 + t + 1) * 128],
                                     xh[ct][:, off_t + c0:off_t + c0 + cw],
                                     start=(t == 0), stop=(t == 8))
                nc.scalar.activation(val[ct][:, c0:c0 + cw], ps[:, :cw], AF.Copy)

    vtab_writes = []
    PCH = [(c * 128, min(128, PIXF - c * 128)) for c in range((PIXF + 127) // 128)]
    ident16 = pool.tile([128, 128], F16, tag="ident16")
    dma.dma_start(ident16[:], io["ident16"][:, :])
    with tc.tile_pool(name="vt_ps", bufs=2, space="PSUM") as vtp, \
         tc.tile_pool(name="vt_sb", bufs=3) as vts:
        for p0, pw in PCH:
            ps = vtp.tile([128, C], F16, tag="vt_ps", space="PSUM")
            for ct in range(CT):
                nc.tensor.transpose(ps[:pw, ct * 128:(ct + 1) * 128],
                                    val[ct][:, p0:p0 + pw], ident16[:, :])
            sb = vts.tile([128, C], F16, tag="vt_sb")
            nc.scalar.activation(sb[:pw, :], ps[:pw, :], AF.Copy)
            wi = dma.dma_start(io["vtab"][p0:p0 + pw, :], sb[:pw, :])
            vtab_writes.append(wi)

    # ---------------- gathers (dma_gather, 4 neighbors) ----------------
    g = []
    for j in range(4):
        t = pool.tile([128, NCH * C], F16, tag=f"g_{j}")
        gi = nc.gpsimd.dma_gather(
            out_ap=t[:].rearrange("p (k c) -> p k c", k=NCH),
            in_ap=io["vtab"][:, :],
            idxs_ap=idxw[:, j * 56:j * 56 + 49],
            num_idxs=N, num_idxs_reg=N, elem_size=C, queue_num=j)
        for wi in vtab_writes:
            tile.add_dep_helper(gi.ins, wi.ins, reason="vtab RAW")
        g.append(t)

    # bilinear combine, point-major: vs[p, k, c]
    vs = pool.tile([128, NCH * C], F16, tag="vs")
    for k in range(NCH):
        kn = min(128, N - k * 128)
        sl = slice(k * C, (k + 1) * C)
        nc.vector.tensor_scalar(out=vs[:kn, sl], in0=g[0][:kn, sl],
                                scalar1=wts[:kn, k:k + 1], scalar2=None, op0=MULT)
        for j in range(1, 4):
            nc.vector.scalar_tensor_tensor(out=vs[:kn, sl], in0=g[j][:kn, sl],
                                           scalar=wts[:kn, j * 7 + k:j * 7 + k + 1],
                                           in1=vs[:kn, sl], op0=MULT, op1=ADD)

    # ---------------- key transpose + ksum ----------------
    kT = []
    with tc.tile_pool(name="kt_ps", bufs=2, space="PSUM") as ktp:
        for k in range(NCH):
            kn = min(128, N - k * 128)
            ps = ktp.tile([128, C], F16, tag="kt_ps", space="PSUM")
            for ct in range(CT):
                nc.tensor.transpose(ps[:kn, ct * 128:(ct + 1) * 128],
                                    key[ct][:, k * 128:k * 128 + kn], ident16[:, :])
            t = pool.tile([128, C], F16, tag=f"kT_{k}")
            nc.scalar.activation(t[:kn, :], ps[:kn, :], AF.Copy)
            kT.append(t)
    ksumT = pool.tile([1, C], F32, tag="ksumT")
    with tc.tile_pool(name="ks_ps", bufs=1, space="PSUM") as ksp:
        ks_ps = ksp.tile([1, 128], F32, tag="ks_ps")
        for ct in range(CT):
            r = pool.tile([128, 1], F32, tag=f"ksum_{ct}")
            nc.vector.tensor_reduce(out=r[:], in_=key[ct][:, :],
                                    axis=mybir.AxisListType.X, op=ADD)
            nc.tensor.transpose(ks_ps[:, :], r[:], identF[:, :])
            nc.scalar.activation(ksumT[:, ct * 128:(ct + 1) * 128], ks_ps[:, :], AF.Copy)

    # ---------------- M (per-ct block-diag heads) ----------------
    m16 = []
    with tc.tile_pool(name="m_ps", bufs=1, space="PSUM") as mps:
        for ct in range(CT):
            m_ps = mps.tile([128, 128], F32, tag=f"m_ps{ct}", name=f"m_ps{ct}")
            vsv = vs[:].rearrange("p (k c) -> p k c", k=NCH)
            for k in range(NCH):
                kn = min(128, N - k * 128)
                nc.tensor.matmul(m_ps[:, :], vsv[:kn, k, ct * 128:(ct + 1) * 128],
                                 kT[k][:kn, ct * 128:(ct + 1) * 128],
                                 start=(k == 0), stop=False)
            nc.tensor.matmul(m_ps[:, :], bvT[:, ct * 128:(ct + 1) * 128],
                             ksumT[:, ct * 128:(ct + 1) * 128],
                             start=False, stop=True)
            t = pool.tile([128, 128], F16, tag=f"m16_{ct}")
            nc.scalar.activation(t[:], m_ps[:], AF.Copy, scale=SCALE)
            m16.append(t)

    # ---------------- AT = blockdiag(M)^T W_out^T ----------------
    at16 = []
    with tc.tile_pool(name="at_ps", bufs=1, space="PSUM") as atp:
        for ct in range(CT):
            at_ps = atp.tile([128, C], F32, tag=f"at_ps{ct}", name=f"at_ps{ct}")
            for j in range(4):
                sl = slice(j * 32, (j + 1) * 32)
                nc.tensor.matmul(at_ps[sl, :], m16[ct][sl, sl], wot16[ct][sl, :],
                                 start=True, stop=True, tile_position=(j * 32, j * 32))
            t = pool.tile([128, C], F16, tag=f"at16_{ct}")
            nc.scalar.activation(t[:], at_ps[:], AF.Copy)
            at16.append(t)

    # ---------------- query conv (DVE, flat) ----------------
    q16 = []
    for ct in range(CT):
        t = pool.tile([128, QF], F16, tag=f"q_{ct}")
        flat_taps(nc.vector, t[:], xq[ct], wp[ct], 0, wp[ct], 27, 0, QF)
        q16.append(t)

    # ---------------- y = AT^T @ Q ----------------
    RPC = 7                     # image rows per output chunk
    CW = RPC * W                # 392
    with tc.tile_pool(name="y_ps", bufs=2, space="PSUM") as yps, \
         tc.tile_pool(name="y_sb", bufs=3) as ysb:
        for ot in range(CT):
            for pc in range(HALF_ROWS // RPC):
                y_ps = yps.tile([128, CW], F32, tag="y_ps", space="PSUM")
                for ct in range(CT):
                    qv = q16[ct][:].rearrange("p (h w) -> p h w", h=HALF_ROWS)
                    nc.tensor.matmul(y_ps[:], at16[ct][:, ot * 128:(ot + 1) * 128],
                                     qv[:, pc * RPC:(pc + 1) * RPC, 0:W],
                                     start=(ct == 0), stop=(ct == CT - 1))
                y_sb = ysb.tile([128, CW], F32, tag="y_sb")
                nc.scalar.activation(y_sb[:], y_ps[:], AF.Copy)
                dma.dma_start(io["y"][ot * 128:(ot + 1) * 128, pc * CW:(pc + 1) * CW],
                              y_sb[:])


def build_program():
    if "nc" in _CACHE:
        return _CACHE["nc"]
    nc = bacc.Bacc("TRN2", target_bir_lowering=False, debug=False, num_devices=8,
                   num_swdge_queues=4)
    io = {}
    io["xh"] = nc.dram_tensor("xh", (C, XW), F16, kind="ExternalInput").ap()
    io["xq"] = nc.dram_tensor("xq", (C, QXW), F16, kind="ExternalInput").ap()
    io["dv"] = nc.dram_tensor("dv", (128, 57 * 128), F16, kind="ExternalInput").ap()
    io["wpack"] = nc.dram_tensor("wpack", (C, 32), F32, kind="ExternalInput").ap()
    io["w2t"] = nc.dram_tensor("w2t", (C, 2), F32, kind="ExternalInput").ap()
    io["wot"] = nc.dram_tensor("wot", (C, C), F16, kind="ExternalInput").ap()
    io["bvT"] = nc.dram_tensor("bvT", (1, C), F32, kind="ExternalInput").ap()
    io["refyx"] = nc.dram_tensor("refyx", (2, N), F32, kind="ExternalInput").ap()
    io["identF"] = nc.dram_tensor("identF", (128, 128), F32, kind="ExternalInput").ap()
    io["ident16"] = nc.dram_tensor("ident16", (128, 128), F16, kind="ExternalInput").ap()
    io["vtab"] = nc.dram_tensor("vtab", (PIXF, C), F16).ap()
    io["y"] = nc.dram_tensor("y", (C, HALF_PIX), F32, kind="ExternalOutput").ap()

    with tile.TileContext(nc) as tc:
        with contextlib.ExitStack() as ctx:
            _emit(nc, tc, ctx, io)
    nc.compile()
    _CACHE["nc"] = nc
    return nc


def host_prep(inputs):
    """Build the 8 per-core input maps from full inputs."""
    x = np.asarray(inputs["x"], np.float32)          # (B, C, H, W)
    xpad = np.zeros((B, C, XW), np.float32)
    xpad_img = np.pad(x, ((0, 0), (0, 0), (1, 1), (1, 1)))  # (B, C, 58, 58)
    xpad[:, :, :HP * HP] = xpad_img.reshape(B, C, HP * HP)

    wv = np.asarray(inputs["w_v"], np.float32).reshape(C, 9)
    wk = np.asarray(inputs["w_k"], np.float32).reshape(C, 9)
    bk = np.asarray(inputs["b_k"], np.float32)
    dv = np.zeros((128, 57 * 128), np.float16)
    for ct in range(CT):
        for t in range(9):
            blk = (ct * 9 + t) * 128
            dv[np.arange(128), blk + np.arange(128)] = wv[ct * 128:(ct + 1) * 128, t]
            blk = (27 + ct * 9 + t) * 128
            dv[np.arange(128), blk + np.arange(128)] = wk[ct * 128:(ct + 1) * 128, t]
    for ct in range(CT):
        blk = (54 + ct) * 128
        dv[np.arange(128), blk + np.arange(128)] = bk[ct * 128:(ct + 1) * 128]

    wpack = np.zeros((C, 32), np.float32)
    wpack[:, 0:9] = np.asarray(inputs["w_q"], np.float32).reshape(C, 9)
    wpack[:, 9:18] = np.asarray(inputs["w_k"], np.float32).reshape(C, 9)
    wpack[:, 18:27] = np.asarray(inputs["w_off1"], np.float32).reshape(C, 9)
    wpack[:, 27] = np.asarray(inputs["b_q"], np.float32)
    wpack[:, 28] = np.asarray(inputs["b_k"], np.float32)
    wpack[:, 29] = np.asarray(inputs["b_off1"], np.float32)
    wpack[:, 30] = np.asarray(inputs["ln_g"], np.float32)
    wpack[:, 31] = np.asarray(inputs["ln_b"], np.float32)

    shared = {
        "dv": dv,
        "wpack": wpack,
        "w2t": np.ascontiguousarray(np.asarray(inputs["w_off2"], np.float32).T),
        "wot": np.ascontiguousarray(np.asarray(inputs["w_out"], np.float32).T).astype(np.float16),
        "bvT": np.asarray(inputs["b_v"], np.float32).reshape(1, C),
        "identF": np.eye(128, dtype=np.float32),
        "ident16": np.eye(128, dtype=np.float16),
    }
    ry = (np.arange(KH, dtype=np.float32) + 0.5) / KH * 2 - 1
    rx = (np.arange(KW, dtype=np.float32) + 0.5) / KW * 2 - 1
    shared["refyx"] = np.ascontiguousarray(
        np.stack([np.repeat(ry, KW), np.tile(rx, KH)]), dtype=np.float32)

    in_maps = []
    xh16 = [np.ascontiguousarray(xpad[b]).astype(np.float16) for b in range(B)]
    for core in range(8):
        b, half = core // 2, core % 2
        m = dict(shared)
        m["xh"] = xh16[b]
        r0 = half * HALF_ROWS
        xqs = np.zeros((C, QXW), np.float16)
        xqs[:, :QXW - 2] = xh16[b][:, r0 * HP:r0 * HP + QXW - 2]
        m["xq"] = xqs
        in_maps.append(m)
    return in_maps


def assemble(results):
    y = np.empty((B, C, H, W), np.float32)
    for core in range(8):
        b, half = core // 2, core % 2
        part = results[core]["y"].reshape(C, HALF_ROWS, W)
        y[b, :, half * HALF_ROWS:(half + 1) * HALF_ROWS, :] = part
    return y


def run(inputs, trace=False):
    nc = build_program()
    in_maps = host_prep(inputs)
    res = run_bass_kernel_spmd(nc, in_maps, core_ids=list(range(8)), trace=trace)
    return assemble(res.results), res


def kernel(**inputs):
    out, _ = run(inputs, trace=False)
    return out
